# revision 5
# baseline (speedup 1.0000x reference)
"""GNO message-passing kernel for Trainium2 (8 NeuronCores, edge-parallel) — v3.

Math (matches the reference):
    h  = relu(relu(relu(ea@W1+b1)@W2+b2)@W3+b3)
    w  = (h@W4+b4).reshape(E,16,16)
    msg= einsum('ei,eio->eo', x[src], w)
    agg= segment_mean(msg, dst, N)
    out= x@root + agg + bias

v3 structure (on-device aggregation, minimal per-tile instruction count):
  - Edges sharded 8 ways, sorted by dst; 128-node windows, G groups of 128
    edges per window (G uniform so one SPMD program serves all cores).
  - Each loop iteration processes TWO windows (2G groups): one attr DMA,
    one meta DMA pair, one device-built one-hot (is_equal vs iota const),
    2G indirect gathers of x[src] from a replicated x table, MLP on
    TensorE, pair-fused einsum on VectorE (two groups share one PSUM
    bank), G-matmul PSUM accumulation per window, one [256,16] table write.
  - ReduceScatter sums the 8 per-core [N,16] tables; each core finalizes
    its slice: divide by host-precomputed 1/cnt, add x@root + bias.
"""

import math
import numpy as np
import ml_dtypes

import concourse.bass as bass
import concourse.bacc as bacc
import concourse.mybir as mybir
import concourse.tile as tile
from concourse.bass_utils import run_bass_kernel_spmd

BF16 = ml_dtypes.bfloat16

N_NODES = 50000
N_EDGES = 800000
N_CORES = 8
P = 128
NW = 392                    # 128-node windows (incl. padding windows)
N_PAD = NW * P              # 50176
NSLICE = N_PAD // N_CORES   # 6272 nodes per core
NCHUNK = NSLICE // P        # 49 finalize chunks per core
ESH = N_EDGES // N_CORES    # 100000 edges per core


# ----------------------------------------------------------------- host prep

def _pack_shard(src, dst, attr, G):
    """Sort by dst, pack each 128-node window's edges into G groups of 128.
    Returns attrT bf16 [8,Ep], msrc i32 [NW/2,128,2G], slotv bf16 same."""
    order = np.argsort(dst, kind="stable")
    src, dst, attr = src[order], dst[order], attr[order]
    w = dst >> 7
    cnt_w = np.bincount(w, minlength=NW)
    assert cnt_w.max() <= G * P
    starts = np.zeros(NW + 1, np.int64)
    starts[1:] = np.cumsum(cnt_w)
    pos = np.arange(len(dst)) - starts[w]
    WT = G * P
    flat = w * WT + pos
    Ep = NW * WT
    src_p = np.zeros(Ep, np.int32)
    slot_p = np.full(Ep, -1.0, np.float32)
    attr_p = np.zeros((Ep, 8), np.float32)
    src_p[flat] = src
    slot_p[flat] = dst & 127
    attr_p[flat] = attr
    # per-iteration meta: [NW//2, 128, 2G] with group j = (window w%2)*G + g
    msrc = np.ascontiguousarray(
        src_p.reshape(NW // 2, 2 * G, P).transpose(0, 2, 1))
    slotv = np.ascontiguousarray(
        slot_p.reshape(NW // 2, 2 * G, P).transpose(0, 2, 1)).astype(BF16)
    attrT = np.ascontiguousarray(attr_p.T).astype(BF16)
    return attrT, msrc, slotv


def _prep_inputs(x, edge_index, edge_attr, W1, b1, W2, b2, W3, b3, W4, b4,
                 root, bias):
    src_all = np.asarray(edge_index[0], np.int64).astype(np.int32)
    dst_all = np.asarray(edge_index[1], np.int64).astype(np.int32)
    attr_all = np.asarray(edge_attr, np.float32)

    maxc = 0
    for k in range(N_CORES):
        d = dst_all[k * ESH:(k + 1) * ESH]
        maxc = max(maxc, np.bincount(d >> 7, minlength=NW).max())
    G = max(2, math.ceil(maxc / P))

    W4p = np.asarray(W4, np.float32).reshape(100, 16, 16).transpose(0, 2, 1).reshape(100, 256)
    b4p = np.asarray(b4, np.float32).reshape(16, 16).T.reshape(256)
    W4a = np.concatenate([W4p, b4p[None, :]], axis=0).astype(BF16)  # [101,256]
    roota = np.concatenate([np.asarray(root, np.float32),
                            np.asarray(bias, np.float32)[None, :]], axis=0).astype(BF16)
    W3a = np.concatenate([np.asarray(W3, np.float32),
                          np.zeros((100, 1), np.float32)], axis=1).astype(BF16)
    b3a = np.concatenate([np.asarray(b3, np.float32),
                          np.ones(1, np.float32)]).reshape(101, 1)
    iota = np.ascontiguousarray(
        np.tile(np.arange(P, dtype=np.float32), (P, 2 * G))).astype(BF16)

    xp = np.zeros((N_PAD, 16), np.float32)
    xp[:N_NODES] = np.asarray(x, np.float32)
    xb = np.ascontiguousarray(xp.astype(BF16))
    cnt = np.bincount(dst_all, minlength=N_PAD).astype(np.float32)
    recip = 1.0 / np.maximum(cnt, 1.0)

    const = {
        "W1": np.asarray(W1, np.float32).astype(BF16),
        "W2": np.asarray(W2, np.float32).astype(BF16),
        "W3": W3a,
        "W4a": W4a,
        "b1": np.asarray(b1, np.float32).reshape(100, 1),
        "b2": np.asarray(b2, np.float32).reshape(100, 1),
        "b3": b3a,
        "roota": roota,
        "iota": iota,
        "xrep": xb,
    }

    in_maps = []
    for k in range(N_CORES):
        sl = slice(k * ESH, (k + 1) * ESH)
        attrT, msrc, slotv = _pack_shard(src_all[sl], dst_all[sl], attr_all[sl], G)
        xsl = xb[k * NSLICE:(k + 1) * NSLICE]
        xslT = np.ascontiguousarray(
            np.concatenate([xsl.T.astype(np.float32),
                            np.ones((1, NSLICE), np.float32)], axis=0)).astype(BF16)
        recipT = np.ascontiguousarray(
            recip[k * NSLICE:(k + 1) * NSLICE].reshape(NCHUNK, P).T)  # [128,49]
        in_maps.append(dict(const, attrT=attrT, msrc=msrc, slotv=slotv,
                            xslT=xslT, recipT=recipT))
    return in_maps, G


# ------------------------------------------------------------ device program

_PROG_CACHE = {}


def build_program(G):
    if G in _PROG_CACHE:
        return _PROG_CACHE[G]

    f32, bf16, i32 = mybir.dt.float32, mybir.dt.bfloat16, mybir.dt.int32
    WT = G * P
    G2 = 2 * G
    WT2 = 2 * WT
    Ep = NW * WT
    NT = NW // 2

    nc = bacc.Bacc(None, target_bir_lowering=False, debug=False, num_devices=N_CORES)
    attrT = nc.dram_tensor("attrT", [8, Ep], bf16, kind="ExternalInput")
    msrc = nc.dram_tensor("msrc", [NT, P, G2], i32, kind="ExternalInput")
    slotv = nc.dram_tensor("slotv", [NT, P, G2], bf16, kind="ExternalInput")
    xrep = nc.dram_tensor("xrep", [N_PAD, 16], bf16, kind="ExternalInput")
    xslT = nc.dram_tensor("xslT", [17, NSLICE], bf16, kind="ExternalInput")
    recipT = nc.dram_tensor("recipT", [P, NCHUNK], f32, kind="ExternalInput")
    W1 = nc.dram_tensor("W1", [8, 100], bf16, kind="ExternalInput")
    W2 = nc.dram_tensor("W2", [100, 100], bf16, kind="ExternalInput")
    W3 = nc.dram_tensor("W3", [100, 101], bf16, kind="ExternalInput")
    W4a = nc.dram_tensor("W4a", [101, 256], bf16, kind="ExternalInput")
    b1 = nc.dram_tensor("b1", [100, 1], f32, kind="ExternalInput")
    b2 = nc.dram_tensor("b2", [100, 1], f32, kind="ExternalInput")
    b3 = nc.dram_tensor("b3", [101, 1], f32, kind="ExternalInput")
    roota = nc.dram_tensor("roota", [17, 16], bf16, kind="ExternalInput")
    iota = nc.dram_tensor("iota", [P, WT2], bf16, kind="ExternalInput")
    out = nc.dram_tensor("out", [NSLICE, 16], f32, kind="ExternalOutput")

    table = nc.dram_tensor("table", [N_PAD, 16], f32, kind="Internal")
    rs_out = nc.dram_tensor("rs_out", [NSLICE, 16], f32, kind="Internal")

    AT = mybir.ActivationFunctionType
    AX = mybir.AxisListType
    OP = mybir.AluOpType
    RG = [list(range(N_CORES))]

    with tile.TileContext(nc) as tc, \
         nc.allow_low_precision(reason="bf16 intermediates, fp32 accumulation"):
        with tc.tile_pool(name="consts", bufs=1) as cp, \
             tc.tile_pool(name="work", bufs=4) as wp, \
             tc.tile_pool(name="psmlp", bufs=3, space="PSUM") as pm, \
             tc.tile_pool(name="psw", bufs=3, space="PSUM") as pw, \
             tc.tile_pool(name="psagg", bufs=2, space="PSUM") as pa:

            W1sb = cp.tile([8, 100], bf16)
            W2sb = cp.tile([100, 100], bf16)
            W3sb = cp.tile([100, 101], bf16)
            W4sb = cp.tile([101, 256], bf16)
            b1sb = cp.tile([100, 1], f32)
            b2sb = cp.tile([100, 1], f32)
            b3sb = cp.tile([101, 1], f32)
            rsb = cp.tile([17, 16], bf16)
            iosb = cp.tile([P, WT2], bf16)
            xtsb = cp.tile([17, NSLICE], bf16)
            rcsb = cp.tile([P, NCHUNK], f32)
            for t_sb, t_dr in ((W1sb, W1), (W2sb, W2), (W3sb, W3), (W4sb, W4a),
                               (b1sb, b1), (b2sb, b2), (b3sb, b3), (rsb, roota),
                               (iosb, iota), (xtsb, xslT), (rcsb, recipT)):
                nc.sync.dma_start(t_sb[:], t_dr[:])

            npair = G2 // 2
            for t in range(NT):
                a_sb = wp.tile([8, WT2], bf16, tag="attr")
                nc.sync.dma_start(a_sb[:], attrT[:, t * WT2:(t + 1) * WT2])
                ms = wp.tile([P, G2], i32, tag="msrc")
                nc.sync.dma_start(ms[:], msrc[t])
                sv = wp.tile([P, G2], bf16, tag="slotv")
                nc.sync.dma_start(sv[:], slotv[t])

                oh = wp.tile([P, WT2], bf16, tag="oh")
                nc.vector.tensor_tensor(
                    out=oh[:].rearrange("p (g s) -> p g s", s=P),
                    in0=sv[:, :, None].to_broadcast([P, G2, P]),
                    in1=iosb[:].rearrange("p (g s) -> p g s", s=P),
                    op=OP.is_equal)

                xg = wp.tile([P, G2, 16], bf16, tag="xg")
                for g in range(G2):
                    nc.gpsimd.indirect_dma_start(
                        out=xg[:, g, :], out_offset=None, in_=xrep[:],
                        in_offset=bass.IndirectOffsetOnAxis(ap=ms[:, g:g + 1], axis=0))

                h3 = wp.tile([101, WT2], bf16, tag="h3")
                for half in range(2):
                    hs = slice(half * WT, (half + 1) * WT)
                    ps1 = pm.tile([100, WT], f32, tag="mlp")
                    nc.tensor.matmul(ps1[:], lhsT=W1sb[:], rhs=a_sb[:, hs],
                                     start=True, stop=True)
                    h1 = wp.tile([100, WT], bf16, tag="h1")
                    nc.scalar.activation(h1[:], ps1[:], AT.Relu, bias=b1sb[:, 0:1])
                    ps2 = pm.tile([100, WT], f32, tag="mlp")
                    nc.tensor.matmul(ps2[:], lhsT=W2sb[:], rhs=h1[:],
                                     start=True, stop=True)
                    h2 = wp.tile([100, WT], bf16, tag="h2")
                    nc.scalar.activation(h2[:], ps2[:], AT.Relu, bias=b2sb[:, 0:1])
                    ps3 = pm.tile([101, WT], f32, tag="mlp")
                    nc.tensor.matmul(ps3[:], lhsT=W3sb[:], rhs=h2[:],
                                     start=True, stop=True)
                    nc.scalar.activation(h3[:, hs], ps3[:], AT.Relu, bias=b3sb[:, 0:1])

                mt = wp.tile([P, G2, 16], bf16, tag="mt")
                for pair in range(npair):
                    g0 = 2 * pair
                    wps = pw.tile([P, 2, 256], f32, tag="w")
                    for d in range(2):
                        g = g0 + d
                        nc.tensor.matmul(wps[:, d, :], lhsT=h3[:, g * P:(g + 1) * P],
                                         rhs=W4sb[:], start=True, stop=True)
                    pr = wp.tile([P, 2, 256], bf16, tag="prod")
                    nc.vector.tensor_tensor(
                        out=pr[:].rearrange("p d (o i) -> p d o i", i=16),
                        in0=wps[:].rearrange("p d (o i) -> p d o i", i=16),
                        in1=xg[:, g0:g0 + 2, None, :].to_broadcast([P, 2, 16, 16]),
                        op=OP.mult)
                    nc.vector.reduce_sum(
                        out=mt[:, g0:g0 + 2, :],
                        in_=pr[:].rearrange("p d (o i) -> p d o i", i=16), axis=AX.X)

                scat = wp.tile([P, 2, 16], f32, tag="scat")
                for w in range(2):
                    ag = pa.tile([P, 16], f32, tag="agg")
                    for g in range(G):
                        gg = w * G + g
                        nc.tensor.matmul(ag[:], lhsT=oh[:, gg * P:(gg + 1) * P],
                                         rhs=mt[:, gg, :],
                                         start=(g == 0), stop=(g == G - 1))
                    nc.scalar.copy(scat[:, w, :], ag[:])
                nc.sync.dma_start(
                    table[t * 2 * P:(t + 1) * 2 * P, :].rearrange(
                        "(w p) o -> p w o", w=2), scat[:])

            nc.gpsimd.collective_compute(
                "ReduceScatter", OP.add, replica_groups=RG,
                ins=[table[:]], outs=[rs_out[:]])

            # finalize: out = rs/cnt + x@root + bias  (node-major chunks)
            tg = wp.tile([P, NCHUNK, 16], f32, tag="tagg")
            nc.sync.dma_start(tg[:], rs_out[:].rearrange("(c p) i -> p c i", p=P))
            ts = wp.tile([P, NCHUNK, 16], f32, tag="tsc")
            nc.vector.tensor_tensor(
                out=ts[:], in0=tg[:],
                in1=rcsb[:, :, None].to_broadcast([P, NCHUNK, 16]), op=OP.mult)
            rw = wp.tile([P, NCHUNK, 16], f32, tag="rootw")
            for c in range(NCHUNK):
                rp = pa.tile([P, 16], f32, tag="agg")
                nc.tensor.matmul(rp[:], lhsT=xtsb[:, c * P:(c + 1) * P],
                                 rhs=rsb[:], start=True, stop=True)
                nc.scalar.copy(rw[:, c, :], rp[:])
            ot = wp.tile([P, NCHUNK, 16], f32, tag="ot")
            nc.vector.tensor_tensor(out=ot[:], in0=rw[:], in1=ts[:], op=OP.add)
            nc.sync.dma_start(out[:].rearrange("(c p) i -> p c i", p=P), ot[:])

    nc.compile()
    _PROG_CACHE[G] = nc
    return nc


# ------------------------------------------------------------------- driver

def _assemble(outs):
    """outs: [n_cores, NSLICE, 16] -> [N_NODES, 16] f32."""
    return np.concatenate(list(outs), axis=0)[:N_NODES].astype(np.float32)


def _run(inputs, trace=False):
    in_maps, G = _prep_inputs(**inputs)
    nc = build_program(G)
    res = run_bass_kernel_spmd(nc, in_maps, list(range(N_CORES)), trace=trace)
    out = _assemble([r["out"] for r in res.results])
    return out, res


def kernel(**inputs) -> np.ndarray:
    out, _ = _run(inputs, trace=False)
    return out
